# revision 32
# baseline (speedup 1.0000x reference)
"""Trainium2 Bass kernel for nn_Complex_Concat_Layer.

res[b,i,j,c] = s[b,c,i]·(v1+v3) + e[b,c,j]·(v2-v3) + sum_h s[b,c,i,h]·v4[h]·e[b,c,j,h]
output layout [B, L, L, C] (channel innermost).

Sharding: channel-parallel — core k computes channel c=k for both batches
over the full LxL span; every input byte is read by exactly one core.

Device computes ONLY the rank-H product m[i,j] = sum_h sv[i,h]·e[j,h] with
sv = v4*s, using fp8(e4m3) DoubleRow matmuls (K=256 per instruction, 2x PE
throughput vs fp16 — measured at the 157 TF/s fp8 peak). The rank-1 terms
a[i] = s·(v1+v3) and b[j] = e·(v2-v3) are computed host-side in f32 and
added during reassembly — keeping them out of the fp8 path cuts the
quantization error from ~2.6e-2 to ~1.5e-2 (fro), under the 2e-2 gate.

Per core schedule (it-outer, jh pairs interleaved across two PSUM banks so
each LDWEIGHTS serves two matmuls): 64 DoubleRow matmuls, 32 PSUM->SBUF
f16 copies alternating ScalarE (ACTIVATE Copy) / VectorE (tensor_copy),
256 KiB per-i-tile output stores on Sync issued as soon as each half is
evacuated. Batch-0 input tiles load on three parallel queues (sync,
scalar, gpsimd); batch-1 follows on gpsimd. 16 short dummy matmuls on a
zeroed tile keep the PE busy from the end of the NEFF preamble until the
inputs land, so the HAM p-state ramp (~3.4us of continuous execution to
reach 2.4 GHz) completes before the real stream starts — the real 64-MM
stream then runs gap-free at the fp8 peak (~216 ns per 128x256x512
DoubleRow matmul).
"""

import sys

if "/opt/trn_rl_repo" not in sys.path:
    sys.path.insert(0, "/opt/trn_rl_repo")

from contextlib import ExitStack

import ml_dtypes
import numpy as np

import concourse.bass as bass
import concourse.mybir as mybir
import concourse.tile as tile
from concourse import bacc
from concourse.bass_utils import run_bass_kernel_spmd

B, C, L, H = 2, 8, 1024, 512
N_CORES = 8
IT = 8           # i tiles of 128 (full L)
HT = 4           # h tiles of 128
JH = 2           # j halves of 512
SG = 2           # sv chunks per batch (4 i-tiles each)

F32 = mybir.dt.float32
F16 = mybir.dt.float16
F8 = mybir.dt.float8e4
NP_F8 = ml_dtypes.float8_e4m3  # TRN FP8_EXP4: bias 7, max normal 240
DR = mybir.MatmulPerfMode.DoubleRow


def build_nc(reps=1, warmups=16, warm_cols=256, pre_warmups=3):
    nc = bacc.Bacc("TRN2", target_bir_lowering=False, debug=False,
                   num_devices=N_CORES)

    # [b, sg, p, ht*512 + il*128 + c]: h = ht*128+p, i = sg*512+il*128+c
    sv_d = nc.dram_tensor("sv", [B, SG, 128, HT * 512], F8, kind="ExternalInput")
    # [b, jh, p, ht*512 + (j - jh*512)]
    e_d = nc.dram_tensor("e", [B, JH, 128, HT * 512], F8, kind="ExternalInput")
    # [b*4 + itp, p, u*1024 + j]: i = itp*256 + u*128 + p
    o_d = nc.dram_tensor("o", [B * IT // 2, 128, 2 * L], F16, kind="ExternalOutput")

    # Pre-TileContext phase — raw (non-tile) resources and instructions
    # that run ahead of the tile-entry synchronization:
    #
    # 1. Batch-0 input loads on three parallel DMA queues, issued as raw
    #    dma_starts so the triggers execute right after each engine's
    #    preamble (~0.7us earlier than in-context triggers). Each DMA
    #    increments `insem` by 16 on COMPLETION (one per DMA engine —
    #    same convention as concourse/benchmark/consecutive_dma_perf.py).
    # 2. All HAM warmup matmuls (see below) on a raw SBUF region into a
    #    dedicated raw PSUM bank (side="right"; the tile pool uses only 7
    #    banks, so no collision).
    # 3. A single PE wait for insem >= 64 (all four loads complete) after
    #    the warmups. Every in-context PE instruction is behind this wait
    #    in program order, so the real matmuls can never race the raw
    #    input buffers regardless of how the tile scheduler orders them.
    #
    # The HAM clock-gates the PE after idle and needs ~3.5us of continuous
    # execution to reach full clock; real matmuls pay a ~2x cold penalty
    # until then. The warmup count is sized so the PE drains them right
    # when the inputs have landed and the ramp has completed.
    warm = nc.alloc_sbuf_tensor("warm_sb", [128, 2, 512], F8, side="right")
    wps = nc.alloc_psum_tensor("warm_ps", [128, 512], F32, side="right")
    e_raw = [
        nc.alloc_sbuf_tensor(f"e{jh}_raw", [128, HT, 512], F8, side="right")
        for jh in range(JH)
    ]
    sv_raw = [
        nc.alloc_sbuf_tensor(f"sv{sg}_raw", [128, HT, 512], F8, side="right")
        for sg in range(SG)
    ]
    insem = nc.alloc_semaphore("b0_in_sem")
    nc.sync.dma_start(out=e_raw[0][:, :, :], in_=e_d[0, 0]).then_inc(insem, 16)
    nc.scalar.dma_start(out=e_raw[1][:, :, :], in_=e_d[0, 1]).then_inc(insem, 16)
    for sg in range(SG):
        nc.gpsimd.dma_start(
            out=sv_raw[sg][:, :, :], in_=sv_d[0, sg]
        ).then_inc(insem, 16)

    def warm_mm():
        nc.tensor.matmul(
            wps[:, :warm_cols], lhsT=warm[:, :, :128],
            rhs=warm[:, :, :warm_cols],
            start=True, stop=True, perf_mode=DR,
        )

    for w in range(pre_warmups + warmups):
        warm_mm()
    nc.tensor.wait_ge(insem, 64)

    with tile.TileContext(nc) as tc, ExitStack() as ctx:
        sv_pool = ctx.enter_context(tc.tile_pool(name="sv", bufs=2))
        e_pool = ctx.enter_context(tc.tile_pool(name="e", bufs=2))
        ot_pool = ctx.enter_context(tc.tile_pool(name="ot", bufs=4))
        pmm = ctx.enter_context(tc.tile_pool(name="pmm", bufs=7, space="PSUM"))

        for rep in range(reps):
            # Batch-1 inputs stay tile-based on GpSimd — needed ~8us in.
            et_b1 = [
                e_pool.tile([128, HT, 512], F8, tag="e", name=f"e_{rep}_{jh}")
                for jh in range(JH)
            ]
            svt_b1 = [
                sv_pool.tile([128, HT, 512], F8, tag="sv", name=f"sv_{rep}_{sg}")
                for sg in range(SG)
            ]
            for jh in range(JH):
                nc.gpsimd.dma_start(out=et_b1[jh], in_=e_d[1, jh])
            for sg in range(SG):
                nc.gpsimd.dma_start(out=svt_b1[sg], in_=sv_d[1, sg])

            ncopy = 0
            for b in range(B):
                et = e_raw if b == 0 else et_b1
                svt = sv_raw if b == 0 else svt_b1
                for itp in range(IT // 2):
                    ot = ot_pool.tile([128, 2 * L], F16, tag="ot",
                                      name=f"ot_{rep}_{b}_{itp}")
                    for u in range(2):
                        it = itp * 2 + u
                        sg, il = it // 4, it % 4
                        pms = [
                            pmm.tile([128, 512], F32, tag="pmm",
                                     name=f"pm_{rep}_{b}_{it}_{jh}")
                            for jh in range(JH)
                        ]
                        # two PSUM banks accumulate jh0/jh1 in parallel
                        # so each LDWEIGHTS serves two matmuls
                        for g in range(HT // 2):
                            lhsT = svt[sg][:, 2 * g:2 * g + 2,
                                           il * 128:(il + 1) * 128]
                            for jh in range(JH):
                                nc.tensor.matmul(
                                    pms[jh],
                                    lhsT=lhsT,
                                    rhs=et[jh][:, 2 * g:2 * g + 2, :],
                                    start=(g == 0),
                                    stop=(g == HT // 2 - 1),
                                    perf_mode=DR,
                                )
                        for jh in range(JH):
                            dst = ot[:, u * L + jh * 512:u * L + (jh + 1) * 512]
                            if ncopy % 2 == 0:
                                nc.scalar.copy(out=dst, in_=pms[jh])
                            else:
                                nc.vector.tensor_copy(out=dst, in_=pms[jh])
                            ncopy += 1
                        # store each 256 KiB i-tile half as soon as its two
                        # evacuations land — spreads output DMA through the
                        # stream and leaves only 256 KiB draining at the end
                        row = b * (IT // 2) + itp
                        nc.sync.dma_start(out=o_d[row][:, u * L:(u + 1) * L],
                                          in_=ot[:, u * L:(u + 1) * L])

    nc.compile()
    return nc


def make_in_maps(start_hidden, end_hidden, v):
    s = np.asarray(start_hidden, dtype=np.float32)
    e = np.asarray(end_hidden, dtype=np.float32)
    v = np.asarray(v, dtype=np.float32)

    v4 = v[3 * H:]
    sv = s * v4[None, None, None, :]  # [B, C, L, H]

    in_maps = []
    for k in range(N_CORES):
        # [B, H, L] transposed operands for core k's channel
        x = np.ascontiguousarray(sv[:, k].transpose(0, 2, 1))
        # -> [b, ht, p, sg, il, c] -> [b, sg, p, (ht, il, c)]
        x = x.reshape(B, HT, 128, SG, 4, 128).transpose(0, 3, 2, 1, 4, 5)
        sv_pack = np.ascontiguousarray(x.reshape(B, SG, 128, HT * 512)).astype(NP_F8)

        y = np.ascontiguousarray(e[:, k].transpose(0, 2, 1))
        y = y.reshape(B, HT, 128, JH, 512).transpose(0, 3, 2, 1, 4)
        e_pack = np.ascontiguousarray(y.reshape(B, JH, 128, HT * 512)).astype(NP_F8)

        in_maps.append({"sv": sv_pack, "e": e_pack})
    return in_maps


def _host_bias(start_hidden, end_hidden, v):
    s = np.asarray(start_hidden, dtype=np.float32)
    e = np.asarray(end_hidden, dtype=np.float32)
    v = np.asarray(v, dtype=np.float32)
    w1 = v[:H] + v[2 * H:3 * H]
    w2 = v[H:2 * H] - v[2 * H:3 * H]
    a = s @ w1   # [B, C, L]
    bb = e @ w2  # [B, C, L]
    return a, bb


def _unpack_core(o_core, out, k, a, bb):
    """o_core [B*4, 128, 2048] f16 -> out[:, :, :, k] f32 (+ biases)."""
    for b in range(B):
        x = o_core[b * (IT // 2):(b + 1) * (IT // 2)]  # [4, 128, 2048]
        x = x.reshape(IT // 2, 128, 2, L).transpose(0, 2, 1, 3).reshape(L, L)
        out[b, :, :, k] = (
            x.astype(np.float32)
            + a[b, k][:, None]
            + bb[b, k][None, :]
        )


_NC = None


def _get_nc():
    global _NC
    if _NC is None:
        _NC = build_nc()
    return _NC


def kernel(start_hidden, end_hidden, v):
    in_maps = make_in_maps(start_hidden, end_hidden, v)
    a, bb = _host_bias(start_hidden, end_hidden, v)
    nc = _get_nc()
    res = run_bass_kernel_spmd(nc, in_maps, core_ids=list(range(N_CORES)))

    out = np.empty((B, L, L, C), dtype=np.float32)
    for k in range(N_CORES):
        _unpack_core(res.results[k]["o"], out, k, a, bb)
    return out


# revision 36
# speedup vs baseline: 1.0988x; 1.0988x over previous
"""Trainium2 Bass kernel for nn_Complex_Concat_Layer.

res[b,i,j,c] = s[b,c,i]·(v1+v3) + e[b,c,j]·(v2-v3) + sum_h s[b,c,i,h]·v4[h]·e[b,c,j,h]
output layout [B, L, L, C] (channel innermost).

Sharding: channel-parallel — core k computes channel c=k for both batches
over the full LxL span; every input byte is read by exactly one core.

Device computes ONLY the rank-H product m[i,j] = sum_h sv[i,h]·e[j,h] with
sv = v4*s, using fp8(e4m3) DoubleRow matmuls (K=256 per instruction, 2x PE
throughput vs fp16 — measured at the 157 TF/s fp8 peak). The rank-1 terms
a[i] = s·(v1+v3) and b[j] = e·(v2-v3) are computed host-side in f32 and
added during reassembly — keeping them out of the fp8 path cuts the
quantization error from ~2.6e-2 to ~1.5e-2 (fro), under the 2e-2 gate.

Per core schedule (it-outer, jh pairs interleaved across two PSUM banks so
each LDWEIGHTS serves two matmuls): 64 DoubleRow matmuls, 32 PSUM->SBUF
f16 copies alternating ScalarE (ACTIVATE Copy) / VectorE (tensor_copy),
256 KiB per-i-tile output stores on Sync issued as soon as each half is
evacuated. Batch-0 input tiles load on three parallel queues (sync,
scalar, gpsimd); batch-1 follows on gpsimd. 16 short dummy matmuls on a
zeroed tile keep the PE busy from the end of the NEFF preamble until the
inputs land, so the HAM p-state ramp (~3.4us of continuous execution to
reach 2.4 GHz) completes before the real stream starts — the real 64-MM
stream then runs gap-free at the fp8 peak (~216 ns per 128x256x512
DoubleRow matmul).
"""

import sys

if "/opt/trn_rl_repo" not in sys.path:
    sys.path.insert(0, "/opt/trn_rl_repo")

from contextlib import ExitStack

import ml_dtypes
import numpy as np

import concourse.bass as bass
import concourse.mybir as mybir
import concourse.tile as tile
from concourse import bacc
from concourse.bass_utils import run_bass_kernel_spmd

B, C, L, H = 2, 8, 1024, 512
N_CORES = 8
IT = 8           # i tiles of 128 (full L)
HT = 4           # h tiles of 128
JH = 2           # j halves of 512
SG = 2           # sv chunks per batch (4 i-tiles each)

F32 = mybir.dt.float32
F16 = mybir.dt.float16
F8 = mybir.dt.float8e4
NP_F8 = ml_dtypes.float8_e4m3  # TRN FP8_EXP4: bias 7, max normal 240
DR = mybir.MatmulPerfMode.DoubleRow


def build_nc(reps=1, warmups=16, warm_cols=256, pre_warmups=3):
    nc = bacc.Bacc("TRN2", target_bir_lowering=False, debug=False,
                   num_devices=N_CORES)

    # [b, sg, p, ht*512 + il*128 + c]: h = ht*128+p, i = sg*512+il*128+c
    sv_d = nc.dram_tensor("sv", [B, SG, 128, HT * 512], F8, kind="ExternalInput")
    # [b, jh, p, ht*512 + (j - jh*512)]
    e_d = nc.dram_tensor("e", [B, JH, 128, HT * 512], F8, kind="ExternalInput")
    # [b*4 + itp, p, u*1024 + j]: i = itp*256 + u*128 + p
    o_d = nc.dram_tensor("o", [B * IT // 2, 128, 2 * L], F16, kind="ExternalOutput")

    # Pre-TileContext phase — raw (non-tile) resources and instructions
    # that run ahead of the tile-entry synchronization:
    #
    # 1. Batch-0 input loads on three parallel DMA queues, issued as raw
    #    dma_starts so the triggers execute right after each engine's
    #    preamble (~0.7us earlier than in-context triggers). Each DMA
    #    increments `insem` by 16 on COMPLETION (one per DMA engine —
    #    same convention as concourse/benchmark/consecutive_dma_perf.py).
    # 2. All HAM warmup matmuls (see below) on a raw SBUF region into a
    #    dedicated raw PSUM bank (side="right"; the tile pool uses only 7
    #    banks, so no collision).
    # 3. A single PE wait for insem >= 64 (all four loads complete) after
    #    the warmups. Every in-context PE instruction is behind this wait
    #    in program order, so the real matmuls can never race the raw
    #    input buffers regardless of how the tile scheduler orders them.
    #
    # The HAM clock-gates the PE after idle and needs ~3.5us of continuous
    # execution to reach full clock; real matmuls pay a ~2x cold penalty
    # until then. The warmup count is sized so the PE drains them right
    # when the inputs have landed and the ramp has completed.
    warm = nc.alloc_sbuf_tensor("warm_sb", [128, 2, 512], F8, side="right")
    wps = nc.alloc_psum_tensor("warm_ps", [128, 512], F32, side="right")
    e_raw = [
        nc.alloc_sbuf_tensor(f"e{jh}_raw", [128, HT, 512], F8, side="right")
        for jh in range(JH)
    ]
    sv_raw = [
        nc.alloc_sbuf_tensor(f"sv{sg}_raw", [128, HT, 512], F8, side="right")
        for sg in range(SG)
    ]
    insem = nc.alloc_semaphore("b0_in_sem")
    nc.sync.dma_start(out=e_raw[0][:, :, :], in_=e_d[0, 0]).then_inc(insem, 16)
    nc.scalar.dma_start(out=e_raw[1][:, :, :], in_=e_d[0, 1]).then_inc(insem, 16)
    for sg in range(SG):
        nc.gpsimd.dma_start(
            out=sv_raw[sg][:, :, :], in_=sv_d[0, sg]
        ).then_inc(insem, 16)

    def warm_mm():
        nc.tensor.matmul(
            wps[:, :warm_cols], lhsT=warm[:, :, :128],
            rhs=warm[:, :, :warm_cols],
            start=True, stop=True, perf_mode=DR,
        )

    for w in range(pre_warmups + warmups):
        warm_mm()
    nc.tensor.wait_ge(insem, 64)

    with tile.TileContext(nc) as tc, ExitStack() as ctx:
        sv_pool = ctx.enter_context(tc.tile_pool(name="sv", bufs=2))
        e_pool = ctx.enter_context(tc.tile_pool(name="e", bufs=2))
        ot_pool = ctx.enter_context(tc.tile_pool(name="ot", bufs=4))
        pmm = ctx.enter_context(tc.tile_pool(name="pmm", bufs=7, space="PSUM"))

        for rep in range(reps):
            # Batch-1 inputs stay tile-based on GpSimd — needed ~8us in.
            et_b1 = [
                e_pool.tile([128, HT, 512], F8, tag="e", name=f"e_{rep}_{jh}")
                for jh in range(JH)
            ]
            svt_b1 = [
                sv_pool.tile([128, HT, 512], F8, tag="sv", name=f"sv_{rep}_{sg}")
                for sg in range(SG)
            ]
            for jh in range(JH):
                nc.gpsimd.dma_start(out=et_b1[jh], in_=e_d[1, jh])
            for sg in range(SG):
                nc.gpsimd.dma_start(out=svt_b1[sg], in_=sv_d[1, sg])

            ncopy = 0
            for b in range(B):
                et = e_raw if b == 0 else et_b1
                svt = sv_raw if b == 0 else svt_b1
                for itp in range(IT // 2):
                    ot = ot_pool.tile([128, 2 * L], F16, tag="ot",
                                      name=f"ot_{rep}_{b}_{itp}")
                    for u in range(2):
                        it = itp * 2 + u
                        sg, il = it // 4, it % 4
                        pms = [
                            pmm.tile([128, 512], F32, tag="pmm",
                                     name=f"pm_{rep}_{b}_{it}_{jh}")
                            for jh in range(JH)
                        ]
                        # two PSUM banks accumulate jh0/jh1 in parallel
                        # so each LDWEIGHTS serves two matmuls
                        for g in range(HT // 2):
                            lhsT = svt[sg][:, 2 * g:2 * g + 2,
                                           il * 128:(il + 1) * 128]
                            for jh in range(JH):
                                nc.tensor.matmul(
                                    pms[jh],
                                    lhsT=lhsT,
                                    rhs=et[jh][:, 2 * g:2 * g + 2, :],
                                    start=(g == 0),
                                    stop=(g == HT // 2 - 1),
                                    perf_mode=DR,
                                )
                        for jh in range(JH):
                            dst = ot[:, u * L + jh * 512:u * L + (jh + 1) * 512]
                            if ncopy % 2 == 0:
                                nc.scalar.copy(out=dst, in_=pms[jh])
                            else:
                                nc.vector.tensor_copy(out=dst, in_=pms[jh])
                            ncopy += 1
                        # store each 256 KiB i-tile half as soon as its two
                        # evacuations land — spreads output DMA through the
                        # stream and leaves only 256 KiB draining at the end
                        row = b * (IT // 2) + itp
                        nc.sync.dma_start(out=o_d[row][:, u * L:(u + 1) * L],
                                          in_=ot[:, u * L:(u + 1) * L])

    nc.compile()
    return nc


def make_in_maps(start_hidden, end_hidden, v):
    s = np.asarray(start_hidden, dtype=np.float32)
    e = np.asarray(end_hidden, dtype=np.float32)
    v = np.asarray(v, dtype=np.float32)

    v4 = v[3 * H:]
    sv = s * v4[None, None, None, :]  # [B, C, L, H]

    in_maps = []
    for k in range(N_CORES):
        # [B, H, L] transposed operands for core k's channel
        x = np.ascontiguousarray(sv[:, k].transpose(0, 2, 1))
        # -> [b, ht, p, sg, il, c] -> [b, sg, p, (ht, il, c)]
        x = x.reshape(B, HT, 128, SG, 4, 128).transpose(0, 3, 2, 1, 4, 5)
        sv_pack = np.ascontiguousarray(x.reshape(B, SG, 128, HT * 512)).astype(NP_F8)

        y = np.ascontiguousarray(e[:, k].transpose(0, 2, 1))
        y = y.reshape(B, HT, 128, JH, 512).transpose(0, 3, 2, 1, 4)
        e_pack = np.ascontiguousarray(y.reshape(B, JH, 128, HT * 512)).astype(NP_F8)

        in_maps.append({"sv": sv_pack, "e": e_pack})
    return in_maps


def _host_bias(start_hidden, end_hidden, v):
    s = np.asarray(start_hidden, dtype=np.float32)
    e = np.asarray(end_hidden, dtype=np.float32)
    v = np.asarray(v, dtype=np.float32)
    w1 = v[:H] + v[2 * H:3 * H]
    w2 = v[H:2 * H] - v[2 * H:3 * H]
    a = s @ w1   # [B, C, L]
    bb = e @ w2  # [B, C, L]
    return a, bb


def _unpack_core(o_core, out, k, a, bb):
    """o_core [B*4, 128, 2048] f16 -> out[:, :, :, k] f32 (+ biases)."""
    for b in range(B):
        x = o_core[b * (IT // 2):(b + 1) * (IT // 2)]  # [4, 128, 2048]
        x = x.reshape(IT // 2, 128, 2, L).transpose(0, 2, 1, 3).reshape(L, L)
        out[b, :, :, k] = (
            x.astype(np.float32)
            + a[b, k][:, None]
            + bb[b, k][None, :]
        )


_NC = None


def _get_nc():
    global _NC
    if _NC is None:
        _NC = build_nc()
    return _NC


def kernel(start_hidden, end_hidden, v):
    in_maps = make_in_maps(start_hidden, end_hidden, v)
    a, bb = _host_bias(start_hidden, end_hidden, v)
    nc = _get_nc()
    res = run_bass_kernel_spmd(nc, in_maps, core_ids=list(range(N_CORES)))

    out = np.empty((B, L, L, C), dtype=np.float32)
    for k in range(N_CORES):
        _unpack_core(res.results[k]["o"], out, k, a, bb)
    return out


# revision 37
# speedup vs baseline: 1.1418x; 1.0391x over previous
"""Trainium2 Bass kernel for nn_Complex_Concat_Layer.

res[b,i,j,c] = s[b,c,i]·(v1+v3) + e[b,c,j]·(v2-v3) + sum_h s[b,c,i,h]·v4[h]·e[b,c,j,h]
output layout [B, L, L, C] (channel innermost).

Sharding: channel-parallel — core k computes channel c=k for both batches
over the full LxL span; every input byte is read by exactly one core.

Device computes ONLY the rank-H product m[i,j] = sum_h sv[i,h]·e[j,h] with
sv = v4*s, using fp8(e4m3) DoubleRow matmuls (K=256 per instruction, 2x PE
throughput vs fp16 — measured at the 157 TF/s fp8 peak). The rank-1 terms
a[i] = s·(v1+v3) and b[j] = e·(v2-v3) are computed host-side in f32 and
added during reassembly — keeping them out of the fp8 path cuts the
quantization error from ~2.6e-2 to ~1.5e-2 (fro), under the 2e-2 gate.

Per core schedule (it-outer, jh pairs interleaved across two PSUM banks so
each LDWEIGHTS serves two matmuls): 64 DoubleRow matmuls, 32 PSUM->SBUF
f16 copies alternating ScalarE (ACTIVATE Copy) / VectorE (tensor_copy),
256 KiB per-i-tile output stores on Sync issued as soon as each half is
evacuated. Batch-0 input tiles load on three parallel queues (sync,
scalar, gpsimd); batch-1 follows on gpsimd. 16 short dummy matmuls on a
zeroed tile keep the PE busy from the end of the NEFF preamble until the
inputs land, so the HAM p-state ramp (~3.4us of continuous execution to
reach 2.4 GHz) completes before the real stream starts — the real 64-MM
stream then runs gap-free at the fp8 peak (~216 ns per 128x256x512
DoubleRow matmul).
"""

import sys

if "/opt/trn_rl_repo" not in sys.path:
    sys.path.insert(0, "/opt/trn_rl_repo")

from contextlib import ExitStack

import ml_dtypes
import numpy as np

import concourse.bass as bass
import concourse.mybir as mybir
import concourse.tile as tile
from concourse import bacc
from concourse.bass_utils import run_bass_kernel_spmd

B, C, L, H = 2, 8, 1024, 512
N_CORES = 8
IT = 8           # i tiles of 128 (full L)
HT = 4           # h tiles of 128
JH = 2           # j halves of 512
SG = 2           # sv chunks per batch (4 i-tiles each)

F32 = mybir.dt.float32
F16 = mybir.dt.float16
F8 = mybir.dt.float8e4
NP_F8 = ml_dtypes.float8_e4m3  # TRN FP8_EXP4: bias 7, max normal 240
DR = mybir.MatmulPerfMode.DoubleRow


def build_nc(reps=1, warmups=16, warm_cols=256, pre_warmups=3):
    nc = bacc.Bacc("TRN2", target_bir_lowering=False, debug=False,
                   num_devices=N_CORES)

    # [b, sg, p, ht*512 + il*128 + c]: h = ht*128+p, i = sg*512+il*128+c
    sv_d = nc.dram_tensor("sv", [B, SG, 128, HT * 512], F8, kind="ExternalInput")
    # [b, jh, p, ht*512 + (j - jh*512)]
    e_d = nc.dram_tensor("e", [B, JH, 128, HT * 512], F8, kind="ExternalInput")
    # [b*4 + itp, p, u*1024 + j]: i = itp*256 + u*128 + p
    o_d = nc.dram_tensor("o", [B * IT // 2, 128, 2 * L], F16, kind="ExternalOutput")

    # Pre-TileContext phase — raw (non-tile) resources and instructions
    # that run ahead of the tile-entry synchronization:
    #
    # 1. Batch-0 input loads on three parallel DMA queues, issued as raw
    #    dma_starts so the triggers execute right after each engine's
    #    preamble (~0.7us earlier than in-context triggers). Each DMA
    #    increments `insem` by 16 on COMPLETION (one per DMA engine —
    #    same convention as concourse/benchmark/consecutive_dma_perf.py).
    # 2. All HAM warmup matmuls (see below) on a raw SBUF region into a
    #    dedicated raw PSUM bank (side="right"; the tile pool uses only 7
    #    banks, so no collision).
    # 3. A single PE wait for insem >= 64 (all four loads complete) after
    #    the warmups. Every in-context PE instruction is behind this wait
    #    in program order, so the real matmuls can never race the raw
    #    input buffers regardless of how the tile scheduler orders them.
    #
    # The HAM clock-gates the PE after idle and needs ~3.5us of continuous
    # execution to reach full clock; real matmuls pay a ~2x cold penalty
    # until then. The warmup count is sized so the PE drains them right
    # when the inputs have landed and the ramp has completed.
    warm = nc.alloc_sbuf_tensor("warm_sb", [128, 2, 512], F8, side="right")
    wps = nc.alloc_psum_tensor("warm_ps", [128, 512], F32, side="right")
    e_raw = [
        nc.alloc_sbuf_tensor(f"e{jh}_raw", [128, HT, 512], F8, side="right")
        for jh in range(JH)
    ]
    sv0_raw = nc.alloc_sbuf_tensor("sv0_raw", [128, HT, 512], F8, side="right")
    insem = nc.alloc_semaphore("b0_in_sem")
    nc.sync.dma_start(out=e_raw[0][:, :, :], in_=e_d[0, 0]).then_inc(insem, 16)
    nc.scalar.dma_start(out=e_raw[1][:, :, :], in_=e_d[0, 1]).then_inc(insem, 16)
    nc.gpsimd.dma_start(out=sv0_raw[:, :, :], in_=sv_d[0, 0]).then_inc(insem, 16)

    def warm_mm():
        nc.tensor.matmul(
            wps[:, :warm_cols], lhsT=warm[:, :, :128],
            rhs=warm[:, :, :warm_cols],
            start=True, stop=True, perf_mode=DR,
        )

    for w in range(pre_warmups + warmups):
        warm_mm()
    # e0+e1+sv0 (48 = 3 DMAs x 16); sv1 is a framework-
    # tracked tile below, so its consumers wait automatically.
    nc.tensor.wait_ge(insem, 48)

    with tile.TileContext(nc) as tc, ExitStack() as ctx:
        sv_pool = ctx.enter_context(tc.tile_pool(name="sv", bufs=3))
        e_pool = ctx.enter_context(tc.tile_pool(name="e", bufs=2))
        ot_pool = ctx.enter_context(tc.tile_pool(name="ot", bufs=4))
        pmm = ctx.enter_context(tc.tile_pool(name="pmm", bufs=7, space="PSUM"))

        for rep in range(reps):
            # Batch-1 inputs stay tile-based on GpSimd — needed ~8us in.
            et_b1 = [
                e_pool.tile([128, HT, 512], F8, tag="e", name=f"e_{rep}_{jh}")
                for jh in range(JH)
            ]
            svt_b1 = [
                sv_pool.tile([128, HT, 512], F8, tag="sv", name=f"sv_{rep}_{sg}")
                for sg in range(SG)
            ]
            # sv1 first on gpsimd (gates i-tile 4, ~1us into the stream),
            # then batch-1 (needed ~8us in).
            sv1_t = sv_pool.tile([128, HT, 512], F8, tag="sv", name=f"sv1_{rep}")
            nc.gpsimd.dma_start(out=sv1_t, in_=sv_d[0, 1])
            for jh in range(JH):
                nc.gpsimd.dma_start(out=et_b1[jh], in_=e_d[1, jh])
            for sg in range(SG):
                nc.gpsimd.dma_start(out=svt_b1[sg], in_=sv_d[1, sg])

            ncopy = 0
            for b in range(B):
                et = e_raw if b == 0 else et_b1
                svt = [sv0_raw, sv1_t] if b == 0 else svt_b1
                for itp in range(IT // 2):
                    ot = ot_pool.tile([128, 2 * L], F16, tag="ot",
                                      name=f"ot_{rep}_{b}_{itp}")
                    for u in range(2):
                        it = itp * 2 + u
                        sg, il = it // 4, it % 4
                        pms = [
                            pmm.tile([128, 512], F32, tag="pmm",
                                     name=f"pm_{rep}_{b}_{it}_{jh}")
                            for jh in range(JH)
                        ]
                        # two PSUM banks accumulate jh0/jh1 in parallel
                        # so each LDWEIGHTS serves two matmuls
                        for g in range(HT // 2):
                            lhsT = svt[sg][:, 2 * g:2 * g + 2,
                                           il * 128:(il + 1) * 128]
                            for jh in range(JH):
                                nc.tensor.matmul(
                                    pms[jh],
                                    lhsT=lhsT,
                                    rhs=et[jh][:, 2 * g:2 * g + 2, :],
                                    start=(g == 0),
                                    stop=(g == HT // 2 - 1),
                                    perf_mode=DR,
                                )
                        for jh in range(JH):
                            dst = ot[:, u * L + jh * 512:u * L + (jh + 1) * 512]
                            if ncopy % 2 == 0:
                                nc.scalar.copy(out=dst, in_=pms[jh])
                            else:
                                nc.vector.tensor_copy(out=dst, in_=pms[jh])
                            ncopy += 1
                        # store each 256 KiB i-tile half as soon as its two
                        # evacuations land — spreads output DMA through the
                        # stream and leaves only 256 KiB draining at the end
                        row = b * (IT // 2) + itp
                        nc.sync.dma_start(out=o_d[row][:, u * L:(u + 1) * L],
                                          in_=ot[:, u * L:(u + 1) * L])

    nc.compile()
    return nc


def make_in_maps(start_hidden, end_hidden, v):
    s = np.asarray(start_hidden, dtype=np.float32)
    e = np.asarray(end_hidden, dtype=np.float32)
    v = np.asarray(v, dtype=np.float32)

    v4 = v[3 * H:]
    sv = s * v4[None, None, None, :]  # [B, C, L, H]

    in_maps = []
    for k in range(N_CORES):
        # [B, H, L] transposed operands for core k's channel
        x = np.ascontiguousarray(sv[:, k].transpose(0, 2, 1))
        # -> [b, ht, p, sg, il, c] -> [b, sg, p, (ht, il, c)]
        x = x.reshape(B, HT, 128, SG, 4, 128).transpose(0, 3, 2, 1, 4, 5)
        sv_pack = np.ascontiguousarray(x.reshape(B, SG, 128, HT * 512)).astype(NP_F8)

        y = np.ascontiguousarray(e[:, k].transpose(0, 2, 1))
        y = y.reshape(B, HT, 128, JH, 512).transpose(0, 3, 2, 1, 4)
        e_pack = np.ascontiguousarray(y.reshape(B, JH, 128, HT * 512)).astype(NP_F8)

        in_maps.append({"sv": sv_pack, "e": e_pack})
    return in_maps


def _host_bias(start_hidden, end_hidden, v):
    s = np.asarray(start_hidden, dtype=np.float32)
    e = np.asarray(end_hidden, dtype=np.float32)
    v = np.asarray(v, dtype=np.float32)
    w1 = v[:H] + v[2 * H:3 * H]
    w2 = v[H:2 * H] - v[2 * H:3 * H]
    a = s @ w1   # [B, C, L]
    bb = e @ w2  # [B, C, L]
    return a, bb


def _unpack_core(o_core, out, k, a, bb):
    """o_core [B*4, 128, 2048] f16 -> out[:, :, :, k] f32 (+ biases)."""
    for b in range(B):
        x = o_core[b * (IT // 2):(b + 1) * (IT // 2)]  # [4, 128, 2048]
        x = x.reshape(IT // 2, 128, 2, L).transpose(0, 2, 1, 3).reshape(L, L)
        out[b, :, :, k] = (
            x.astype(np.float32)
            + a[b, k][:, None]
            + bb[b, k][None, :]
        )


_NC = None


def _get_nc():
    global _NC
    if _NC is None:
        _NC = build_nc()
    return _NC


def kernel(start_hidden, end_hidden, v):
    in_maps = make_in_maps(start_hidden, end_hidden, v)
    a, bb = _host_bias(start_hidden, end_hidden, v)
    nc = _get_nc()
    res = run_bass_kernel_spmd(nc, in_maps, core_ids=list(range(N_CORES)))

    out = np.empty((B, L, L, C), dtype=np.float32)
    for k in range(N_CORES):
        _unpack_core(res.results[k]["o"], out, k, a, bb)
    return out


# revision 38
# speedup vs baseline: 1.1646x; 1.0200x over previous
"""Trainium2 Bass kernel for nn_Complex_Concat_Layer.

res[b,i,j,c] = s[b,c,i]·(v1+v3) + e[b,c,j]·(v2-v3) + sum_h s[b,c,i,h]·v4[h]·e[b,c,j,h]
output layout [B, L, L, C] (channel innermost).

Sharding: channel-parallel — core k computes channel c=k for both batches
over the full LxL span; every input byte is read by exactly one core.

Device computes ONLY the rank-H product m[i,j] = sum_h sv[i,h]·e[j,h] with
sv = v4*s, using fp8(e4m3) DoubleRow matmuls (K=256 per instruction, 2x PE
throughput vs fp16 — measured at the 157 TF/s fp8 peak). The rank-1 terms
a[i] = s·(v1+v3) and b[j] = e·(v2-v3) are computed host-side in f32 and
added during reassembly — keeping them out of the fp8 path cuts the
quantization error from ~2.6e-2 to ~1.5e-2 (fro), under the 2e-2 gate.

Per core schedule (it-outer, jh pairs interleaved across two PSUM banks so
each LDWEIGHTS serves two matmuls): 64 DoubleRow matmuls, 32 PSUM->SBUF
f16 copies alternating ScalarE (ACTIVATE Copy) / VectorE (tensor_copy),
256 KiB per-i-tile output stores on Sync issued as soon as each half is
evacuated. Batch-0 input tiles load on three parallel queues (sync,
scalar, gpsimd); batch-1 follows on gpsimd. 16 short dummy matmuls on a
zeroed tile keep the PE busy from the end of the NEFF preamble until the
inputs land, so the HAM p-state ramp (~3.4us of continuous execution to
reach 2.4 GHz) completes before the real stream starts — the real 64-MM
stream then runs gap-free at the fp8 peak (~216 ns per 128x256x512
DoubleRow matmul).
"""

import sys

if "/opt/trn_rl_repo" not in sys.path:
    sys.path.insert(0, "/opt/trn_rl_repo")

from contextlib import ExitStack

import ml_dtypes
import numpy as np

import concourse.bass as bass
import concourse.mybir as mybir
import concourse.tile as tile
from concourse import bacc
from concourse.bass_utils import run_bass_kernel_spmd

B, C, L, H = 2, 8, 1024, 512
N_CORES = 8
IT = 8           # i tiles of 128 (full L)
HT = 4           # h tiles of 128
JH = 2           # j halves of 512
SG = 2           # sv chunks per batch (4 i-tiles each)

F32 = mybir.dt.float32
F16 = mybir.dt.float16
F8 = mybir.dt.float8e4
NP_F8 = ml_dtypes.float8_e4m3  # TRN FP8_EXP4: bias 7, max normal 240
DR = mybir.MatmulPerfMode.DoubleRow


def build_nc(reps=1, warmups=13, warm_cols=256, pre_warmups=3):
    nc = bacc.Bacc("TRN2", target_bir_lowering=False, debug=False,
                   num_devices=N_CORES)

    # [b, sg, p, ht*512 + il*128 + c]: h = ht*128+p, i = sg*512+il*128+c
    sv_d = nc.dram_tensor("sv", [B, SG, 128, HT * 512], F8, kind="ExternalInput")
    # [b, jh, p, ht*512 + (j - jh*512)]
    e_d = nc.dram_tensor("e", [B, JH, 128, HT * 512], F8, kind="ExternalInput")
    # [b*4 + itp, p, u*1024 + j]: i = itp*256 + u*128 + p
    o_d = nc.dram_tensor("o", [B * IT // 2, 128, 2 * L], F16, kind="ExternalOutput")

    # Pre-TileContext phase — raw (non-tile) resources and instructions
    # that run ahead of the tile-entry synchronization:
    #
    # 1. Batch-0 input loads on three parallel DMA queues, issued as raw
    #    dma_starts so the triggers execute right after each engine's
    #    preamble (~0.7us earlier than in-context triggers). Each DMA
    #    increments `insem` by 16 on COMPLETION (one per DMA engine —
    #    same convention as concourse/benchmark/consecutive_dma_perf.py).
    # 2. All HAM warmup matmuls (see below) on a raw SBUF region into a
    #    dedicated raw PSUM bank (side="right"; the tile pool uses only 7
    #    banks, so no collision).
    # 3. A single PE wait for insem >= 64 (all four loads complete) after
    #    the warmups. Every in-context PE instruction is behind this wait
    #    in program order, so the real matmuls can never race the raw
    #    input buffers regardless of how the tile scheduler orders them.
    #
    # The HAM clock-gates the PE after idle and needs ~3.5us of continuous
    # execution to reach full clock; real matmuls pay a ~2x cold penalty
    # until then. The warmup count is sized so the PE drains them right
    # when the inputs have landed and the ramp has completed.
    warm = nc.alloc_sbuf_tensor("warm_sb", [128, 2, 512], F8, side="right")
    wps = nc.alloc_psum_tensor("warm_ps", [128, 512], F32, side="right")
    e_raw = [
        nc.alloc_sbuf_tensor(f"e{jh}_raw", [128, HT, 512], F8, side="right")
        for jh in range(JH)
    ]
    sv0_raw = nc.alloc_sbuf_tensor("sv0_raw", [128, HT, 512], F8, side="right")
    insem = nc.alloc_semaphore("b0_in_sem")
    nc.sync.dma_start(out=e_raw[0][:, :, :], in_=e_d[0, 0]).then_inc(insem, 16)
    nc.scalar.dma_start(out=e_raw[1][:, :, :], in_=e_d[0, 1]).then_inc(insem, 16)
    nc.gpsimd.dma_start(out=sv0_raw[:, :, :], in_=sv_d[0, 0]).then_inc(insem, 16)

    def warm_mm():
        nc.tensor.matmul(
            wps[:, :warm_cols], lhsT=warm[:, :, :128],
            rhs=warm[:, :, :warm_cols],
            start=True, stop=True, perf_mode=DR,
        )

    for w in range(pre_warmups + warmups):
        warm_mm()
    # e0+e1+sv0 (48 = 3 DMAs x 16); sv1 is a framework-
    # tracked tile below, so its consumers wait automatically.
    nc.tensor.wait_ge(insem, 48)

    with tile.TileContext(nc) as tc, ExitStack() as ctx:
        sv_pool = ctx.enter_context(tc.tile_pool(name="sv", bufs=3))
        e_pool = ctx.enter_context(tc.tile_pool(name="e", bufs=2))
        ot_pool = ctx.enter_context(tc.tile_pool(name="ot", bufs=4))
        pmm = ctx.enter_context(tc.tile_pool(name="pmm", bufs=7, space="PSUM"))

        for rep in range(reps):
            # Batch-1 inputs stay tile-based on GpSimd — needed ~8us in.
            et_b1 = [
                e_pool.tile([128, HT, 512], F8, tag="e", name=f"e_{rep}_{jh}")
                for jh in range(JH)
            ]
            svt_b1 = [
                sv_pool.tile([128, HT, 512], F8, tag="sv", name=f"sv_{rep}_{sg}")
                for sg in range(SG)
            ]
            # sv1 first on gpsimd (gates i-tile 4, ~1us into the stream),
            # then batch-1 (needed ~8us in).
            sv1_t = sv_pool.tile([128, HT, 512], F8, tag="sv", name=f"sv1_{rep}")
            nc.gpsimd.dma_start(out=sv1_t, in_=sv_d[0, 1])
            for jh in range(JH):
                nc.gpsimd.dma_start(out=et_b1[jh], in_=e_d[1, jh])
            for sg in range(SG):
                nc.gpsimd.dma_start(out=svt_b1[sg], in_=sv_d[1, sg])

            ncopy = 0
            for b in range(B):
                et = e_raw if b == 0 else et_b1
                svt = [sv0_raw, sv1_t] if b == 0 else svt_b1
                for itp in range(IT // 2):
                    ot = ot_pool.tile([128, 2 * L], F16, tag="ot",
                                      name=f"ot_{rep}_{b}_{itp}")
                    for u in range(2):
                        it = itp * 2 + u
                        sg, il = it // 4, it % 4
                        pms = [
                            pmm.tile([128, 512], F32, tag="pmm",
                                     name=f"pm_{rep}_{b}_{it}_{jh}")
                            for jh in range(JH)
                        ]
                        # two PSUM banks accumulate jh0/jh1 in parallel
                        # so each LDWEIGHTS serves two matmuls
                        for g in range(HT // 2):
                            lhsT = svt[sg][:, 2 * g:2 * g + 2,
                                           il * 128:(il + 1) * 128]
                            for jh in range(JH):
                                nc.tensor.matmul(
                                    pms[jh],
                                    lhsT=lhsT,
                                    rhs=et[jh][:, 2 * g:2 * g + 2, :],
                                    start=(g == 0),
                                    stop=(g == HT // 2 - 1),
                                    perf_mode=DR,
                                )
                        for jh in range(JH):
                            dst = ot[:, u * L + jh * 512:u * L + (jh + 1) * 512]
                            if ncopy % 2 == 0:
                                nc.scalar.copy(out=dst, in_=pms[jh])
                            else:
                                nc.vector.tensor_copy(out=dst, in_=pms[jh])
                            ncopy += 1
                        # store each 256 KiB i-tile half as soon as its two
                        # evacuations land — spreads output DMA through the
                        # stream and leaves only 256 KiB draining at the end
                        row = b * (IT // 2) + itp
                        nc.sync.dma_start(out=o_d[row][:, u * L:(u + 1) * L],
                                          in_=ot[:, u * L:(u + 1) * L])

    nc.compile()
    return nc


def make_in_maps(start_hidden, end_hidden, v):
    s = np.asarray(start_hidden, dtype=np.float32)
    e = np.asarray(end_hidden, dtype=np.float32)
    v = np.asarray(v, dtype=np.float32)

    v4 = v[3 * H:]
    sv = s * v4[None, None, None, :]  # [B, C, L, H]

    in_maps = []
    for k in range(N_CORES):
        # [B, H, L] transposed operands for core k's channel
        x = np.ascontiguousarray(sv[:, k].transpose(0, 2, 1))
        # -> [b, ht, p, sg, il, c] -> [b, sg, p, (ht, il, c)]
        x = x.reshape(B, HT, 128, SG, 4, 128).transpose(0, 3, 2, 1, 4, 5)
        sv_pack = np.ascontiguousarray(x.reshape(B, SG, 128, HT * 512)).astype(NP_F8)

        y = np.ascontiguousarray(e[:, k].transpose(0, 2, 1))
        y = y.reshape(B, HT, 128, JH, 512).transpose(0, 3, 2, 1, 4)
        e_pack = np.ascontiguousarray(y.reshape(B, JH, 128, HT * 512)).astype(NP_F8)

        in_maps.append({"sv": sv_pack, "e": e_pack})
    return in_maps


def _host_bias(start_hidden, end_hidden, v):
    s = np.asarray(start_hidden, dtype=np.float32)
    e = np.asarray(end_hidden, dtype=np.float32)
    v = np.asarray(v, dtype=np.float32)
    w1 = v[:H] + v[2 * H:3 * H]
    w2 = v[H:2 * H] - v[2 * H:3 * H]
    a = s @ w1   # [B, C, L]
    bb = e @ w2  # [B, C, L]
    return a, bb


def _unpack_core(o_core, out, k, a, bb):
    """o_core [B*4, 128, 2048] f16 -> out[:, :, :, k] f32 (+ biases)."""
    for b in range(B):
        x = o_core[b * (IT // 2):(b + 1) * (IT // 2)]  # [4, 128, 2048]
        x = x.reshape(IT // 2, 128, 2, L).transpose(0, 2, 1, 3).reshape(L, L)
        out[b, :, :, k] = (
            x.astype(np.float32)
            + a[b, k][:, None]
            + bb[b, k][None, :]
        )


_NC = None


def _get_nc():
    global _NC
    if _NC is None:
        _NC = build_nc()
    return _NC


def kernel(start_hidden, end_hidden, v):
    in_maps = make_in_maps(start_hidden, end_hidden, v)
    a, bb = _host_bias(start_hidden, end_hidden, v)
    nc = _get_nc()
    res = run_bass_kernel_spmd(nc, in_maps, core_ids=list(range(N_CORES)))

    out = np.empty((B, L, L, C), dtype=np.float32)
    for k in range(N_CORES):
        _unpack_core(res.results[k]["o"], out, k, a, bb)
    return out
